# revision 29
# baseline (speedup 1.0000x reference)
"""Block-Circulant-Matrix Linear kernel for Trainium2 (8 NeuronCores, SPMD).

Reference computation:
    W[r*64+i, q*64+j] = w[r, q, (i-j) % 64]        (dense 1024x1024 from w[16,16,64])
    y = x @ W.T                                    (x: [32768, 1024] f32)

Strategy (data-parallel, per sharding hint):
  - Shard x along tokens across 8 cores (4096 tokens each); replicate w.
  - Per core, y_tile = x_tile @ W.T via TensorE with fp32r (full-rate, reduced
    mantissa) matmuls:
      * lhsT = x-tile transposed on TensorE (PE transpose), rounded to fp32r by
        the ScalarE PSUM->SBUF copy.
      * rhs = the circulant W.T is never materialized densely.  Each
        in-channel chunk c keeps a "skewed" SBUF tile S_c[p=(qh,j), f] =
        w2r2[(2c+qh)*2048 + f + j], where w2r2[q, r, t'] = w[r, q, (63-t')%64]
        is a reversed, doubled, (q,r)-transposed copy of w staged in DRAM.
        The skew (+j per partition) is free in the DMA (partition step 1 over
        DRAM) and keeps each partition a single 7936-byte descriptor (large
        descriptors are what make this expansion fast); a strided rhs access
        pattern [(rr: 128), (ii: 1)] then reads exactly W.T with each
        64-block of the out-dim reversed (ii = 63-i).  The reversal is undone
        for free by a negative-step AP in the VectorE PSUM->SBUF copy of y.
  - All DMAs use large contiguous descriptors; no slow gather anywhere.
"""

import numpy as np

N_CORES = 8
N_TOKENS = 32768
TOK_PER_CORE = N_TOKENS // N_CORES  # 4096
IN_CH = 1024
OUT_CH = 1024
BS = 64
R = OUT_CH // BS  # 16
Q = IN_CH // BS   # 16
KCH = IN_CH // 128  # 8 k-chunks of 128 partitions
S_FREE = (R - 1) * 2 * BS + BS  # 1984: covers max n*1024 + rr*128 + ii (+j via skew)

_CACHE = {}


def build_nc(tok_per_core=TOK_PER_CORE):
    from contextlib import ExitStack

    import concourse.bass as bass
    import concourse.mybir as mybir
    import concourse.tile as tile
    from concourse import bacc
    from concourse.masks import make_identity

    f32 = mybir.dt.float32
    f32r = mybir.dt.float32r
    f16 = mybir.dt.float16

    nc = bacc.Bacc("TRN2", target_bir_lowering=False, debug=False)
    x = nc.dram_tensor("x", [tok_per_core, IN_CH], f32, kind="ExternalInput").ap()
    w = nc.dram_tensor("w", [R, Q, BS], f32, kind="ExternalInput").ap()
    y = nc.dram_tensor("y", [tok_per_core, OUT_CH], f32, kind="ExternalOutput").ap()

    n_tok_tiles = tok_per_core // 128

    def rev_last(ap3):
        """Reverse the last (innermost free) dim of an AP."""
        pairs = [list(p) for p in ap3.ap]
        n = pairs[-1][1]
        assert pairs[-1][0] == 1
        pairs[-1][0] = -1
        return bass.AP(ap3.tensor, ap3.offset + n - 1, pairs)

    with tile.TileContext(nc) as tc, ExitStack() as ctx:
        const_pool = ctx.enter_context(tc.tile_pool(name="const", bufs=1))
        wt_pool = ctx.enter_context(tc.tile_pool(name="wt", bufs=1))
        dram_pool = ctx.enter_context(tc.tile_pool(name="dram", bufs=1, space="DRAM"))
        xb_pool = ctx.enter_context(tc.tile_pool(name="xb", bufs=10))
        xt_sb_pool = ctx.enter_context(tc.tile_pool(name="xt_sb", bufs=10))
        y_sb_pool = ctx.enter_context(tc.tile_pool(name="y_sb", bufs=4))
        xt_ps_pool = ctx.enter_context(tc.tile_pool(name="xt_ps", bufs=2, space="PSUM"))
        y_ps_pool = ctx.enter_context(tc.tile_pool(name="y_ps", bufs=2, space="PSUM"))

        identity = const_pool.tile([128, 128], f32)
        make_identity(nc, identity)

        xbs = {}

        def emit_xb(t, eng):
            xb = xb_pool.tile([128, IN_CH], f32, name=f"xb_{t}", tag="xb")
            eng.dma_start(xb, x[t * 128 : (t + 1) * 128, :])
            xbs[t] = xb

        for t in range(2):
            emit_xb(t, nc.sync)

        # --- stage w2[q, r, u] = f32r(w[r, q, u % 64]) (u in [0,128)) in DRAM.
        # w flat is [(r q) = 256, 64]; two SBUF tiles of [128, 64] (r in
        # [8a, 8a+8)).  The (r,q)->(q,r) reorder and the doubling are fused
        # into the SBUF->DRAM store: dst walks (r_local, q, u-half) to match
        # the source partition order.
        w2r2 = dram_pool.tile([Q, R, 2 * BS], f16)
        if True:
            # one DMA loads all of w: w_all[p, a*64 + s] = w[(a*128+p) row, s]
            w_all = const_pool.tile([128, 2 * BS], f32, name="w_all")
            src_w = bass.AP(w.tensor, 0, [[BS, 128], [128 * BS, 2], [1, BS]])
            nc.sync.dma_start(w_all, src_w)
            w_rall = const_pool.tile([128, 2 * BS], f16, name="w_rall")
            nc.vector.tensor_copy(
                rev_last(w_rall[:, :].rearrange("p (h s) -> p h s", s=BS)),
                w_all[:, :].rearrange("p (h s) -> p h s", s=BS),
            )
            for a in range(2):
                for half in range(2):
                    dst3 = bass.AP(
                        w2r2.tensor,
                        w2r2.offset + a * (R // 2) * 2 * BS + half * BS,
                        [[2 * BS, R // 2], [R * 2 * BS, Q], [1, BS]],
                    )
                    nc.sync.dma_start(dst3, w_rall[:, a * BS : (a + 1) * BS])

        # --- skewed replica tiles S_c[(qh,j), f] = w2r2_flat[(2c+qh)*2048+f+j]
        # (single 7936 B descriptor per partition -> near line rate).  All on
        # sync so the scalar engine stays clear for the xt copies.
        s_tiles = [wt_pool.tile([128, S_FREE], f16, name=f"s_{c}") for c in range(KCH)]

        def emit_s_dma(c):
            s_c = s_tiles[c]
            for qh in range(2):
                src = bass.AP(
                    w2r2.tensor,
                    w2r2.offset + (2 * c + qh) * R * 2 * BS,
                    [[1, BS], [1, S_FREE]],
                )
                nc.sync.dma_start(s_c[qh * BS : (qh + 1) * BS, :], src)

        for c in range(KCH):
            emit_s_dma(c)

        def rhs_ap(c, n):
            s_c = s_tiles[c]
            pstride = s_c[:, :].ap[0][0]
            return bass.AP(
                s_c.tensor,
                s_c.offset + n * (R // 2) * 2 * BS,
                [[pstride, 128], [2 * BS, R // 2], [1, BS]],
            )

        # --- main loop over 128-token tiles, software-pipelined: transposes +
        # PSUM->SBUF rounding copies for tile t are emitted before the matmuls
        # of tile t-depth so the PE never waits on the ScalarE copy.
        xts = {}

        def emit_front(t):
            if t not in xbs:
                # the first tiles ride SWDGE so they bypass the S stream on
                # sync; later tiles queue on sync BEHIND the S stream, which
                # throttles x traffic until the weights are resident.
                emit_xb(t, nc.gpsimd if t < 6 else nc.sync)
            xb = xbs.pop(t)
            xt_ps = xt_ps_pool.tile([128, IN_CH], f32, name=f"xt_ps_{t}", tag="xt_ps")
            for c in range(KCH):
                nc.tensor.transpose(
                    xt_ps[:, c * 128 : (c + 1) * 128],
                    xb[:, c * 128 : (c + 1) * 128],
                    identity,
                )
            xt = xt_sb_pool.tile([128, IN_CH], f16, name=f"xt_{t}", tag="xt")
            nc.scalar.copy(xt[:, 0:512], xt_ps[:, 0:512])
            nc.scalar.copy(xt[:, 512:1024], xt_ps[:, 512:1024])
            xts[t] = xt

        def emit_back(t):
            xt = xts.pop(t)
            y_ps = y_ps_pool.tile([128, OUT_CH], f32, name=f"y_ps_{t}", tag="y_ps")
            for c in range(KCH):
                for n in range(OUT_CH // 512):
                    nc.tensor.matmul(
                        y_ps[:, n * 512 : (n + 1) * 512],
                        lhsT=xt[:, c * 128 : (c + 1) * 128],
                        rhs=rhs_ap(c, n),
                        start=(c == 0),
                        stop=(c == KCH - 1),
                    )
            # copy PSUM->SBUF while un-reversing each 64-block of the out-dim:
            #   y_sb[p, n*512 + rr*64 + (63-ii)] = y_ps[p, n*512 + rr*64 + ii]
            y_sb = y_sb_pool.tile([128, OUT_CH], f32, name=f"y_sb_{t}", tag="y_sb")
            last = t >= n_tok_tiles - 2
            for n in range(2):
                src_ = y_ps[:, n * 512 : (n + 1) * 512].rearrange(
                    "p (r i) -> p r i", i=BS
                )
                dst = rev_last(
                    y_sb[:, n * 512 : (n + 1) * 512].rearrange("p (r i) -> p r i", i=BS)
                )
                if last and n == 1:
                    nc.scalar.copy(dst, src_)
                else:
                    nc.vector.tensor_copy(dst, src_)
            if last:
                nc.scalar.dma_start(y[t * 128 : (t + 1) * 128, 0:512], y_sb[:, 0:512])
                nc.sync.dma_start(y[t * 128 : (t + 1) * 128, 512:1024], y_sb[:, 512:1024])
            else:
                nc.scalar.dma_start(y[t * 128 : (t + 1) * 128, :], y_sb)

        depth = 8
        for t in range(n_tok_tiles + depth):
            if t < n_tok_tiles:
                emit_front(t)
            if t >= depth:
                emit_back(t - depth)

    nc.compile()
    return nc


def get_nc(tok_per_core=TOK_PER_CORE):
    if tok_per_core not in _CACHE:
        _CACHE[tok_per_core] = build_nc(tok_per_core)
    return _CACHE[tok_per_core]


def kernel(x: np.ndarray, w: np.ndarray) -> np.ndarray:
    from concourse.bass_utils import run_bass_kernel_spmd

    x = np.ascontiguousarray(x, dtype=np.float32)
    w = np.ascontiguousarray(w, dtype=np.float32)
    assert x.shape == (N_TOKENS, IN_CH), x.shape
    assert w.shape == (R, Q, BS), w.shape

    nc = get_nc()
    in_maps = [
        {"x": x[i * TOK_PER_CORE : (i + 1) * TOK_PER_CORE], "w": w}
        for i in range(N_CORES)
    ]
    res = run_bass_kernel_spmd(nc, in_maps, core_ids=list(range(N_CORES)))
    return np.concatenate([r["y"] for r in res.results], axis=0)


# revision 31
# speedup vs baseline: 1.0180x; 1.0180x over previous
"""Block-Circulant-Matrix Linear kernel for Trainium2 (8 NeuronCores, SPMD).

Reference computation:
    W[r*64+i, q*64+j] = w[r, q, (i-j) % 64]        (dense 1024x1024 from w[16,16,64])
    y = x @ W.T                                    (x: [32768, 1024] f32)

Strategy (data-parallel, per sharding hint):
  - Shard x along tokens across 8 cores (4096 tokens each); replicate w.
  - Per core, y_tile = x_tile @ W.T via TensorE with fp32r (full-rate, reduced
    mantissa) matmuls:
      * lhsT = x-tile transposed on TensorE (PE transpose), rounded to fp32r by
        the ScalarE PSUM->SBUF copy.
      * rhs = the circulant W.T is never materialized densely.  Each
        in-channel chunk c keeps a "skewed" SBUF tile S_c[p=(qh,j), f] =
        w2r2[(2c+qh)*2048 + f + j], where w2r2[q, r, t'] = w[r, q, (63-t')%64]
        is a reversed, doubled, (q,r)-transposed copy of w staged in DRAM.
        The skew (+j per partition) is free in the DMA (partition step 1 over
        DRAM) and keeps each partition a single 7936-byte descriptor (large
        descriptors are what make this expansion fast); a strided rhs access
        pattern [(rr: 128), (ii: 1)] then reads exactly W.T with each
        64-block of the out-dim reversed (ii = 63-i).  The reversal is undone
        for free by a negative-step AP in the VectorE PSUM->SBUF copy of y.
  - All DMAs use large contiguous descriptors; no slow gather anywhere.
"""

import numpy as np

N_CORES = 8
N_TOKENS = 32768
TOK_PER_CORE = N_TOKENS // N_CORES  # 4096
IN_CH = 1024
OUT_CH = 1024
BS = 64
R = OUT_CH // BS  # 16
Q = IN_CH // BS   # 16
KCH = IN_CH // 128  # 8 k-chunks of 128 partitions
S_FREE = (R - 1) * 2 * BS + BS  # 1984: covers max n*1024 + rr*128 + ii (+j via skew)

_CACHE = {}


def build_nc(tok_per_core=TOK_PER_CORE):
    from contextlib import ExitStack

    import concourse.bass as bass
    import concourse.mybir as mybir
    import concourse.tile as tile
    from concourse import bacc
    from concourse.masks import make_identity

    f32 = mybir.dt.float32
    f32r = mybir.dt.float32r
    f16 = mybir.dt.float16

    nc = bacc.Bacc("TRN2", target_bir_lowering=False, debug=False)
    x = nc.dram_tensor("x", [tok_per_core, IN_CH], f32, kind="ExternalInput").ap()
    w = nc.dram_tensor("w", [R, Q, BS], f32, kind="ExternalInput").ap()
    y = nc.dram_tensor("y", [tok_per_core, OUT_CH], f32, kind="ExternalOutput").ap()

    n_tok_tiles = tok_per_core // 128

    def rev_last(ap3):
        """Reverse the last (innermost free) dim of an AP."""
        pairs = [list(p) for p in ap3.ap]
        n = pairs[-1][1]
        assert pairs[-1][0] == 1
        pairs[-1][0] = -1
        return bass.AP(ap3.tensor, ap3.offset + n - 1, pairs)

    with tile.TileContext(nc) as tc, ExitStack() as ctx:
        const_pool = ctx.enter_context(tc.tile_pool(name="const", bufs=1))
        wt_pool = ctx.enter_context(tc.tile_pool(name="wt", bufs=1))
        dram_pool = ctx.enter_context(tc.tile_pool(name="dram", bufs=1, space="DRAM"))
        xb_pool = ctx.enter_context(tc.tile_pool(name="xb", bufs=10))
        xt_sb_pool = ctx.enter_context(tc.tile_pool(name="xt_sb", bufs=10))
        y_sb_pool = ctx.enter_context(tc.tile_pool(name="y_sb", bufs=6))
        xt_ps_pool = ctx.enter_context(tc.tile_pool(name="xt_ps", bufs=2, space="PSUM"))
        y_ps_pool = ctx.enter_context(tc.tile_pool(name="y_ps", bufs=2, space="PSUM"))

        identity = const_pool.tile([128, 128], f32)
        make_identity(nc, identity)

        xbs = {}

        def emit_xb(t, eng):
            xb = xb_pool.tile([128, IN_CH], f32, name=f"xb_{t}", tag="xb")
            eng.dma_start(xb, x[t * 128 : (t + 1) * 128, :])
            xbs[t] = xb

        for t in range(2):
            emit_xb(t, nc.sync)

        # --- stage w2[q, r, u] = f32r(w[r, q, u % 64]) (u in [0,128)) in DRAM.
        # w flat is [(r q) = 256, 64]; two SBUF tiles of [128, 64] (r in
        # [8a, 8a+8)).  The (r,q)->(q,r) reorder and the doubling are fused
        # into the SBUF->DRAM store: dst walks (r_local, q, u-half) to match
        # the source partition order.
        w_flat = w.rearrange("r q s -> (r q) s")
        w2r2 = dram_pool.tile([Q, R, 2 * BS], f16)
        with tc.high_priority():
            for a in range(2):
                w_sb = const_pool.tile([128, BS], f32, name=f"w_sb_{a}")
                nc.sync.dma_start(w_sb, w_flat[a * 128 : (a + 1) * 128, :])
                w_r = const_pool.tile([128, BS], f16, name=f"w_r_{a}")
                nc.vector.tensor_copy(w_r, rev_last(w_sb[:, :]))
                for half in range(2):
                    dst3 = bass.AP(
                        w2r2.tensor,
                        w2r2.offset + a * (R // 2) * 2 * BS + half * BS,
                        [[2 * BS, R // 2], [R * 2 * BS, Q], [1, BS]],
                    )
                    nc.sync.dma_start(dst3, w_r[:, :])

        # --- skewed replica tiles S_c[(qh,j), f] = w2r2_flat[(2c+qh)*2048+f+j]
        # (single 7936 B descriptor per partition -> near line rate).  All on
        # sync so the scalar engine stays clear for the xt copies.
        s_tiles = [wt_pool.tile([128, S_FREE], f16, name=f"s_{c}") for c in range(KCH)]

        def emit_s_dma(c):
            s_c = s_tiles[c]
            for qh in range(2):
                src = bass.AP(
                    w2r2.tensor,
                    w2r2.offset + (2 * c + qh) * R * 2 * BS,
                    [[1, BS], [1, S_FREE]],
                )
                nc.sync.dma_start(s_c[qh * BS : (qh + 1) * BS, :], src)

        for c in range(KCH):
            emit_s_dma(c)

        def rhs_ap(c, n):
            s_c = s_tiles[c]
            pstride = s_c[:, :].ap[0][0]
            return bass.AP(
                s_c.tensor,
                s_c.offset + n * (R // 2) * 2 * BS,
                [[pstride, 128], [2 * BS, R // 2], [1, BS]],
            )

        # --- main loop over 128-token tiles, software-pipelined: transposes +
        # PSUM->SBUF rounding copies for tile t are emitted before the matmuls
        # of tile t-depth so the PE never waits on the ScalarE copy.
        xts = {}

        def emit_front(t):
            if t not in xbs:
                # the first tiles ride SWDGE so they bypass the S stream on
                # sync; later tiles queue on sync BEHIND the S stream, which
                # throttles x traffic until the weights are resident.
                emit_xb(t, nc.gpsimd if t < 6 else nc.sync)
            xb = xbs.pop(t)
            xt_ps = xt_ps_pool.tile([128, IN_CH], f32, name=f"xt_ps_{t}", tag="xt_ps")
            for c in range(KCH):
                nc.tensor.transpose(
                    xt_ps[:, c * 128 : (c + 1) * 128],
                    xb[:, c * 128 : (c + 1) * 128],
                    identity,
                )
            xt = xt_sb_pool.tile([128, IN_CH], f16, name=f"xt_{t}", tag="xt")
            nc.scalar.copy(xt[:, 0:512], xt_ps[:, 0:512])
            nc.scalar.copy(xt[:, 512:1024], xt_ps[:, 512:1024])
            xts[t] = xt

        def emit_back(t):
            xt = xts.pop(t)
            y_ps = y_ps_pool.tile([128, OUT_CH], f32, name=f"y_ps_{t}", tag="y_ps")
            for c in range(KCH):
                for n in range(OUT_CH // 512):
                    nc.tensor.matmul(
                        y_ps[:, n * 512 : (n + 1) * 512],
                        lhsT=xt[:, c * 128 : (c + 1) * 128],
                        rhs=rhs_ap(c, n),
                        start=(c == 0),
                        stop=(c == KCH - 1),
                    )
            # copy PSUM->SBUF while un-reversing each 64-block of the out-dim:
            #   y_sb[p, n*512 + rr*64 + (63-ii)] = y_ps[p, n*512 + rr*64 + ii]
            y_sb = y_sb_pool.tile([128, OUT_CH], f32, name=f"y_sb_{t}", tag="y_sb")
            last = t >= n_tok_tiles - 2
            for n in range(2):
                src_ = y_ps[:, n * 512 : (n + 1) * 512].rearrange(
                    "p (r i) -> p r i", i=BS
                )
                dst = rev_last(
                    y_sb[:, n * 512 : (n + 1) * 512].rearrange("p (r i) -> p r i", i=BS)
                )
                if last and n == 1:
                    nc.scalar.copy(dst, src_)
                else:
                    nc.vector.tensor_copy(dst, src_)
            if last:
                nc.scalar.dma_start(y[t * 128 : (t + 1) * 128, 0:512], y_sb[:, 0:512])
                nc.sync.dma_start(y[t * 128 : (t + 1) * 128, 512:1024], y_sb[:, 512:1024])
            else:
                nc.scalar.dma_start(y[t * 128 : (t + 1) * 128, :], y_sb)

        depth = 8
        for t in range(n_tok_tiles + depth):
            if t < n_tok_tiles:
                emit_front(t)
            if t >= depth:
                emit_back(t - depth)

    nc.compile()
    return nc


def get_nc(tok_per_core=TOK_PER_CORE):
    if tok_per_core not in _CACHE:
        _CACHE[tok_per_core] = build_nc(tok_per_core)
    return _CACHE[tok_per_core]


def kernel(x: np.ndarray, w: np.ndarray) -> np.ndarray:
    from concourse.bass_utils import run_bass_kernel_spmd

    x = np.ascontiguousarray(x, dtype=np.float32)
    w = np.ascontiguousarray(w, dtype=np.float32)
    assert x.shape == (N_TOKENS, IN_CH), x.shape
    assert w.shape == (R, Q, BS), w.shape

    nc = get_nc()
    in_maps = [
        {"x": x[i * TOK_PER_CORE : (i + 1) * TOK_PER_CORE], "w": w}
        for i in range(N_CORES)
    ]
    res = run_bass_kernel_spmd(nc, in_maps, core_ids=list(range(N_CORES)))
    return np.concatenate([r["y"] for r in res.results], axis=0)


# revision 33
# speedup vs baseline: 1.0195x; 1.0015x over previous
"""Block-Circulant-Matrix Linear kernel for Trainium2 (8 NeuronCores, SPMD).

Reference computation:
    W[r*64+i, q*64+j] = w[r, q, (i-j) % 64]        (dense 1024x1024 from w[16,16,64])
    y = x @ W.T                                    (x: [32768, 1024] f32)

Strategy (data-parallel, per sharding hint):
  - Shard x along tokens across 8 cores (4096 tokens each); replicate w.
  - Per core, y_tile = x_tile @ W.T via TensorE with fp32r (full-rate, reduced
    mantissa) matmuls:
      * lhsT = x-tile transposed on TensorE (PE transpose), rounded to fp32r by
        the ScalarE PSUM->SBUF copy.
      * rhs = the circulant W.T is never materialized densely.  Each
        in-channel chunk c keeps a "skewed" SBUF tile S_c[p=(qh,j), f] =
        w2r2[(2c+qh)*2048 + f + j], where w2r2[q, r, t'] = w[r, q, (63-t')%64]
        is a reversed, doubled, (q,r)-transposed copy of w staged in DRAM.
        The skew (+j per partition) is free in the DMA (partition step 1 over
        DRAM) and keeps each partition a single 7936-byte descriptor (large
        descriptors are what make this expansion fast); a strided rhs access
        pattern [(rr: 128), (ii: 1)] then reads exactly W.T with each
        64-block of the out-dim reversed (ii = 63-i).  The reversal is undone
        for free by a negative-step AP in the VectorE PSUM->SBUF copy of y.
  - All DMAs use large contiguous descriptors; no slow gather anywhere.
"""

import numpy as np

N_CORES = 8
N_TOKENS = 32768
TOK_PER_CORE = N_TOKENS // N_CORES  # 4096
IN_CH = 1024
OUT_CH = 1024
BS = 64
R = OUT_CH // BS  # 16
Q = IN_CH // BS   # 16
KCH = IN_CH // 128  # 8 k-chunks of 128 partitions
S_FREE = (R - 1) * 2 * BS + BS  # 1984: covers max n*1024 + rr*128 + ii (+j via skew)

_CACHE = {}


def build_nc(tok_per_core=TOK_PER_CORE):
    from contextlib import ExitStack

    import concourse.bass as bass
    import concourse.mybir as mybir
    import concourse.tile as tile
    from concourse import bacc
    from concourse.masks import make_identity

    f32 = mybir.dt.float32
    f32r = mybir.dt.float32r
    f16 = mybir.dt.float16

    nc = bacc.Bacc("TRN2", target_bir_lowering=False, debug=False)
    x = nc.dram_tensor("x", [tok_per_core, IN_CH], f32, kind="ExternalInput").ap()
    w = nc.dram_tensor("w", [R, Q, BS], f32, kind="ExternalInput").ap()
    y = nc.dram_tensor("y", [tok_per_core, OUT_CH], f32, kind="ExternalOutput").ap()

    n_tok_tiles = tok_per_core // 128

    def rev_last(ap3):
        """Reverse the last (innermost free) dim of an AP."""
        pairs = [list(p) for p in ap3.ap]
        n = pairs[-1][1]
        assert pairs[-1][0] == 1
        pairs[-1][0] = -1
        return bass.AP(ap3.tensor, ap3.offset + n - 1, pairs)

    with tile.TileContext(nc) as tc, ExitStack() as ctx:
        const_pool = ctx.enter_context(tc.tile_pool(name="const", bufs=1))
        wt_pool = ctx.enter_context(tc.tile_pool(name="wt", bufs=1))
        dram_pool = ctx.enter_context(tc.tile_pool(name="dram", bufs=1, space="DRAM"))
        xb_pool = ctx.enter_context(tc.tile_pool(name="xb", bufs=10))
        xt_sb_pool = ctx.enter_context(tc.tile_pool(name="xt_sb", bufs=10))
        y_sb_pool = ctx.enter_context(tc.tile_pool(name="y_sb", bufs=6))
        xt_ps_pool = ctx.enter_context(tc.tile_pool(name="xt_ps", bufs=2, space="PSUM"))
        y_ps_pool = ctx.enter_context(tc.tile_pool(name="y_ps", bufs=2, space="PSUM"))

        identity = const_pool.tile([128, 128], f32)
        make_identity(nc, identity)

        xbs = {}

        def emit_xb(t, eng):
            xb = xb_pool.tile([128, IN_CH], f32, name=f"xb_{t}", tag="xb")
            eng.dma_start(xb, x[t * 128 : (t + 1) * 128, :])
            xbs[t] = xb

        for t in range(2):
            emit_xb(t, nc.sync)

        # --- stage w2[q, r, u] = f32r(w[r, q, u % 64]) (u in [0,128)) in DRAM.
        # w flat is [(r q) = 256, 64]; two SBUF tiles of [128, 64] (r in
        # [8a, 8a+8)).  The (r,q)->(q,r) reorder and the doubling are fused
        # into the SBUF->DRAM store: dst walks (r_local, q, u-half) to match
        # the source partition order.
        w_flat = w.rearrange("r q s -> (r q) s")
        w2r2 = dram_pool.tile([Q, R, 2 * BS], f16)
        with tc.high_priority():
            for a in range(2):
                w_sb = const_pool.tile([128, BS], f32, name=f"w_sb_{a}")
                nc.sync.dma_start(w_sb, w_flat[a * 128 : (a + 1) * 128, :])
                w_r = const_pool.tile([128, BS], f16, name=f"w_r_{a}")
                nc.vector.tensor_copy(w_r, rev_last(w_sb[:, :]))
                for half in range(2):
                    dst3 = bass.AP(
                        w2r2.tensor,
                        w2r2.offset + a * (R // 2) * 2 * BS + half * BS,
                        [[2 * BS, R // 2], [R * 2 * BS, Q], [1, BS]],
                    )
                    nc.sync.dma_start(dst3, w_r[:, :])

        # --- skewed replica tiles S_c[(qh,j), f] = w2r2_flat[(2c+qh)*2048+f+j]
        # (single 7936 B descriptor per partition -> near line rate).  All on
        # sync so the scalar engine stays clear for the xt copies.
        s_tiles = [wt_pool.tile([128, S_FREE], f16, name=f"s_{c}") for c in range(KCH)]

        def emit_s_dma(c):
            s_c = s_tiles[c]
            for qh in range(2):
                src = bass.AP(
                    w2r2.tensor,
                    w2r2.offset + (2 * c + qh) * R * 2 * BS,
                    [[1, BS], [1, S_FREE]],
                )
                nc.sync.dma_start(s_c[qh * BS : (qh + 1) * BS, :], src)

        for c in range(KCH):
            emit_s_dma(c)

        def rhs_ap(c, n):
            s_c = s_tiles[c]
            pstride = s_c[:, :].ap[0][0]
            return bass.AP(
                s_c.tensor,
                s_c.offset + n * (R // 2) * 2 * BS,
                [[pstride, 128], [2 * BS, R // 2], [1, BS]],
            )

        # --- main loop over 128-token tiles, software-pipelined: transposes +
        # PSUM->SBUF rounding copies for tile t are emitted before the matmuls
        # of tile t-depth so the PE never waits on the ScalarE copy.
        xts = {}

        def emit_front(t):
            if t not in xbs:
                # the first tiles ride SWDGE so they bypass the S stream on
                # sync; later tiles queue on sync BEHIND the S stream, which
                # throttles x traffic until the weights are resident.
                emit_xb(t, nc.gpsimd if t < 6 else nc.sync)
            xb = xbs.pop(t)
            xt_ps = xt_ps_pool.tile([128, IN_CH], f32, name=f"xt_ps_{t}", tag="xt_ps")
            for c in range(KCH):
                nc.tensor.transpose(
                    xt_ps[:, c * 128 : (c + 1) * 128],
                    xb[:, c * 128 : (c + 1) * 128],
                    identity,
                )
            xt = xt_sb_pool.tile([128, IN_CH], f16, name=f"xt_{t}", tag="xt")
            nc.scalar.copy(xt[:, 0:512], xt_ps[:, 0:512])
            nc.scalar.copy(xt[:, 512:1024], xt_ps[:, 512:1024])
            xts[t] = xt

        def emit_back(t):
            xt = xts.pop(t)
            y_ps = y_ps_pool.tile([128, OUT_CH], f32, name=f"y_ps_{t}", tag="y_ps")
            for c in range(KCH):
                for n in range(OUT_CH // 512):
                    nc.tensor.matmul(
                        y_ps[:, n * 512 : (n + 1) * 512],
                        lhsT=xt[:, c * 128 : (c + 1) * 128],
                        rhs=rhs_ap(c, n),
                        start=(c == 0),
                        stop=(c == KCH - 1),
                    )
            # copy PSUM->SBUF while un-reversing each 64-block of the out-dim:
            #   y_sb[p, n*512 + rr*64 + (63-ii)] = y_ps[p, n*512 + rr*64 + ii]
            y_sb = y_sb_pool.tile([128, OUT_CH], f32, name=f"y_sb_{t}", tag="y_sb")
            last = t >= n_tok_tiles - 2
            for n in range(2):
                src_ = y_ps[:, n * 512 : (n + 1) * 512].rearrange(
                    "p (r i) -> p r i", i=BS
                )
                dst = rev_last(
                    y_sb[:, n * 512 : (n + 1) * 512].rearrange("p (r i) -> p r i", i=BS)
                )
                if last and n == 1:
                    nc.scalar.copy(dst, src_)
                else:
                    nc.vector.tensor_copy(dst, src_)
            if last:
                nc.scalar.dma_start(y[t * 128 : (t + 1) * 128, 0:512], y_sb[:, 0:512])
                nc.sync.dma_start(y[t * 128 : (t + 1) * 128, 512:1024], y_sb[:, 512:1024])
            else:
                nc.scalar.dma_start(y[t * 128 : (t + 1) * 128, :], y_sb)

        depth = 8
        for t in range(n_tok_tiles + depth):
            if t < n_tok_tiles:
                emit_front(t)
            if t >= depth:
                emit_back(t - depth)

    nc.compile()
    return nc


def get_nc(tok_per_core=TOK_PER_CORE):
    if tok_per_core not in _CACHE:
        _CACHE[tok_per_core] = build_nc(tok_per_core)
    return _CACHE[tok_per_core]


def kernel(x: np.ndarray, w: np.ndarray) -> np.ndarray:
    from concourse.bass_utils import run_bass_kernel_spmd

    x = np.ascontiguousarray(x, dtype=np.float32)
    w = np.ascontiguousarray(w, dtype=np.float32)
    assert x.shape == (N_TOKENS, IN_CH), x.shape
    assert w.shape == (R, Q, BS), w.shape

    nc = get_nc()
    in_maps = [
        {"x": x[i * TOK_PER_CORE : (i + 1) * TOK_PER_CORE], "w": w}
        for i in range(N_CORES)
    ]
    res = run_bass_kernel_spmd(nc, in_maps, core_ids=list(range(N_CORES)))
    return np.concatenate([r["y"] for r in res.results], axis=0)
